# revision 12
# baseline (speedup 1.0000x reference)
"""Trainium2 Bass kernel for the 2-layer CGConv GNN (nn_Net_90881507983728).

Self-contained: host-side prep (sort/shard/pack) + Bass program + SPMD run.

Strategy (8 cores):
  - Edges sorted by dst; nodes split into 8 contiguous ranges balanced by
    padded-degree; per core, nodes packed into 128-slot columns of 8 fixed
    16-row cells (degree padded to multiple of 16).
  - Per conv layer: PE computes node tables [A'|C'] (dst projections + bias)
    and [B|D] (src projections) for all nodes; per edge chunk: dst-side values
    expanded from per-cell gathers by a static PE matmul; src-side values
    fetched by batched indirect DMA gather; DVE/ACT compute
    sigmoid(.)*softplus(.); a static block-ones PE matmul produces per-cell
    sums; per-node sums assembled by a small gather+add; AllGather exchanges
    per-core aggregate slices.
  - Pad/dummy slots have src pointing at a poisoned table row (-30000) so
    their messages are exactly 0.
"""
import numpy as np

import concourse.bass as bass
import concourse.bacc as bacc
import concourse.mybir as mybir
import concourse.tile as tile
from concourse.bass_utils import run_bass_kernel_spmd

N_CORES = 8
CELL = 16
CPC = 8            # cells per column
NCOLS = 51         # columns per chunk (51*10 = 510 <= 512 PSUM free)
CH = 5
F32 = mybir.dt.float32
I32 = mybir.dt.int32
POISON = -30000.0

AF = mybir.ActivationFunctionType


# ----------------------------------------------------------------------------
# host prep
# ----------------------------------------------------------------------------
class P:
    pass


def _prepare(edge_index, edge_attr, n_nodes):
    p = P()
    src = edge_index[0].astype(np.int64)
    dst = edge_index[1].astype(np.int64)
    e = edge_attr.reshape(-1).astype(np.float32)

    deg = np.bincount(dst, minlength=n_nodes).astype(np.int64)
    assert deg.max() <= 128, f"max degree {deg.max()} > 128 unsupported"
    ncells_all = (deg + CELL - 1) // CELL          # 0 cells for deg-0 nodes
    pad_deg = ncells_all * CELL

    cum = np.cumsum(pad_deg)
    total = cum[-1]
    bounds = [0]
    for k in range(1, N_CORES):
        bounds.append(int(np.searchsorted(cum, total * k / N_CORES)))
    bounds.append(n_nodes)

    order = np.argsort(dst, kind="stable")
    src_s = src[order]
    e_s = e[order]
    starts = np.zeros(n_nodes + 1, dtype=np.int64)
    np.cumsum(deg, out=starts[1:])

    # dense packing: each core concatenates its nodes' cells in node order
    # (cells may straddle column boundaries; compact only needs consecutive
    # cell ids per node)
    max_cols = 0
    max_nodes = 0
    for k in range(N_CORES):
        n0, n1 = bounds[k], bounds[k + 1]
        ncell_k = int(ncells_all[n0:n1].sum())
        max_cols = max(max_cols, (ncell_k + CPC - 1) // CPC)
        max_nodes = max(max_nodes, n1 - n0)

    CHUNKS = (max_cols + NCOLS - 1) // NCOLS
    C_pad = CHUNKS * NCOLS
    SP = ((max_nodes + 639) // 640) * 640   # multiple of 128 and 5
    K2 = SP // 128
    N_TOT = SP * N_CORES
    NG = N_TOT // 5
    ZROW = N_TOT
    NCELL_TOT = C_pad * CPC
    ZCELL = NCELL_TOT

    p.CHUNKS, p.C_pad, p.SP, p.K2 = CHUNKS, C_pad, SP, K2
    p.N_TOT, p.NG, p.ZROW, p.NCELL_TOT, p.ZCELL = N_TOT, NG, ZROW, NCELL_TOT, ZCELL
    p.NCELL512 = ((NCELL_TOT + 511) // 512) * 512

    orig2rel = np.zeros(n_nodes, dtype=np.int64)
    for k in range(N_CORES):
        n0, n1 = bounds[k], bounds[k + 1]
        orig2rel[n0:n1] = k * SP + np.arange(n1 - n0)
    p.core = []
    for k in range(N_CORES):
        n0, n1 = bounds[k], bounds[k + 1]
        nodes = np.arange(n0, n1)
        nn = len(nodes)
        lid = np.arange(nn)
        ncl = ncells_all[nodes]                       # cells per node
        cellstart = np.zeros(nn + 1, dtype=np.int64)
        np.cumsum(ncl, out=cellstart[1:])
        ncell_k = int(cellstart[-1])

        # per-edge slot fill (slots linear: cell*CELL + s)
        d_k = deg[nodes]
        tot_e = int(d_k.sum())
        estart = np.zeros(nn + 1, dtype=np.int64)
        np.cumsum(d_k, out=estart[1:])
        ln = np.repeat(lid, d_k)
        r = np.arange(tot_e) - np.repeat(estart[:-1], d_k)
        gpos = np.repeat(starts[nodes], d_k) + r
        slot = cellstart[ln] * CELL + r

        sidx_lin = np.full(NCELL_TOT * CELL, -1, dtype=np.int64)
        e_lin = np.zeros(NCELL_TOT * CELL, dtype=np.float32)
        sidx_lin[slot] = src_s[gpos]
        e_lin[slot] = e_s[gpos]
        # [C_pad*CPC*CELL] -> [128, C_pad]: col = slot//128, row = slot%128
        src_g = np.ascontiguousarray(sidx_lin.reshape(C_pad, 128).T)
        e_g = np.ascontiguousarray(e_lin.reshape(C_pad, 128).T)

        cn_lin = np.full(NCELL_TOT, ZROW, dtype=np.int64)
        cn_lin[:ncell_k] = k * SP + np.repeat(lid, ncl)
        cellnode_g = np.ascontiguousarray(
            cn_lin.reshape(C_pad, CPC).T.astype(np.int32))

        fc_n = np.full(SP, ZCELL, dtype=np.int64)
        fc_n[:nn] = np.where(ncl > 0, cellstart[:-1], ZCELL)
        ncl_full = np.zeros(SP, dtype=np.int64)
        ncl_full[:nn] = ncl

        c = {}
        mask = src_g >= 0
        sg2 = np.full(src_g.shape, ZROW, dtype=np.int32)
        sg2[mask] = orig2rel[src_g[mask]]
        c["src_g"] = sg2
        c["e_g"] = e_g
        c["cellnode_g"] = cellnode_g
        # cell-linear node list + agg-gather idx (celltab2)
        cl512 = np.concatenate([cn_lin, np.full(p.NCELL512 - NCELL_TOT, ZROW,
                                                np.int64)])
        c["cell_lin"] = cl512
        safe = np.where(cl512 >= N_TOT, 0, cl512).astype(np.int32)
        c["aggidx"] = np.ascontiguousarray(safe.reshape(-1, 128).T)
        c["fc"] = fc_n.reshape(128, K2).astype(np.int32)
        m = (np.arange(CPC)[None, :] < ncl_full[:, None]).astype(np.float32)
        c["cmask"] = np.ascontiguousarray(
            np.repeat(m, 5, axis=1).reshape(128, K2 * CPC * 5))
        p.core.append(c)
    p.orig2rel = orig2rel
    return p


def _pack_T(values, p, width):
    """[N_TOT, width] node-major -> [5*width, NG] with partition (grp*width+f)."""
    v = values[: p.N_TOT].reshape(p.NG, 5, width)
    return np.ascontiguousarray(
        np.transpose(v, (1, 2, 0)).reshape(5 * width, p.NG).astype(np.float32))


# ----------------------------------------------------------------------------
# bass program
# ----------------------------------------------------------------------------
def _build(p, debug=False):
    NG, SP, K2, N_TOT = p.NG, p.SP, p.K2, p.N_TOT
    CHUNKS, C_pad, NCELL_TOT = p.CHUNKS, p.C_pad, p.NCELL_TOT
    NGC = (NG + 511) // 512

    nc = bacc.Bacc("TRN2", target_bir_lowering=False, num_devices=N_CORES)

    # inputs
    xT = nc.dram_tensor("xT", [10, NG], F32, kind="ExternalInput")
    lhsT1 = nc.dram_tensor("lhsT1", [10, 125], F32, kind="ExternalInput")
    bias1 = nc.dram_tensor("bias1", [1, 125], F32, kind="ExternalInput")
    lhsT2 = nc.dram_tensor("lhsT2", [25, 100], F32, kind="ExternalInput")
    bias2 = nc.dram_tensor("bias2", [1, 100], F32, kind="ExternalInput")
    lhsTo = nc.dram_tensor("lhsTo", [25, 10], F32, kind="ExternalInput")
    biaso = nc.dram_tensor("biaso", [1, 10], F32, kind="ExternalInput")
    srcg = nc.dram_tensor("srcg", [128, C_pad], I32, kind="ExternalInput")
    eg = nc.dram_tensor("eg", [128, C_pad], F32, kind="ExternalInput")
    NCELL512 = p.NCELL512
    NCIT = NCELL512 // 512
    NCALL = NCELL512 // 128
    xcTP = nc.dram_tensor("xcTP", [8, NCELL512 // 4], F32, kind="ExternalInput")
    lhsTc = nc.dram_tensor("lhsTc", [8, 80], F32, kind="ExternalInput")
    biasc = nc.dram_tensor("biasc", [1, 80], F32, kind="ExternalInput")
    lexp = nc.dram_tensor("lexp", [CPC, 128], F32, kind="ExternalInput")
    wmix2 = nc.dram_tensor("wmix2", [128, 50], F32, kind="ExternalInput")
    aggidx = nc.dram_tensor("aggidx", [128, NCALL], I32, kind="ExternalInput")
    fcd = nc.dram_tensor("fcd", [128, K2], I32, kind="ExternalInput")
    cmask = nc.dram_tensor("cmask", [128, K2 * CPC * 5], F32, kind="ExternalInput")
    wpat = nc.dram_tensor("wpat", [128, 20], F32, kind="ExternalInput")
    lseg = nc.dram_tensor("lseg", [128, 8], F32, kind="ExternalInput")
    onesd = nc.dram_tensor("onesd", [1, 512], F32, kind="ExternalInput")
    out = nc.dram_tensor("out", [N_TOT, 2], F32, kind="ExternalOutput")
    if debug:
        dbg_ch = {nm: nc.dram_tensor(f"dbg_{nm}", [128, NCOLS * 10], F32, kind="ExternalOutput")
                  for nm in ["acexp", "bdg"]}
        dbg_ch2 = {nm: nc.dram_tensor(f"dbg_{nm}", [128, NCOLS * 5], F32, kind="ExternalOutput")
                   for nm in ["ew", "tf", "ts", "gf", "gs", "msg"]}
        dbg_h1 = nc.dram_tensor("dbg_h1", [N_TOT, 5], F32, kind="ExternalOutput")
        dbg_ac = nc.dram_tensor("dbg_ac", [N_TOT + 1, 10], F32, kind="ExternalOutput")
        dbg_bd = nc.dram_tensor("dbg_bd", [N_TOT + 1, 10], F32, kind="ExternalOutput")
        dbg_agc = nc.dram_tensor("dbg_agc", [NCELL_TOT + 1, 5], F32, kind="ExternalOutput")
        dbg_ago = nc.dram_tensor("dbg_ago", [N_TOT, 5], F32, kind="ExternalOutput")
        dbg_ac2 = nc.dram_tensor("dbg_ac2", [N_TOT + 1, 10], F32, kind="ExternalOutput")
        dbg_bd2 = nc.dram_tensor("dbg_bd2", [N_TOT + 1, 10], F32, kind="ExternalOutput")
        dbg_ago2 = nc.dram_tensor("dbg_ago2", [N_TOT, 5], F32, kind="ExternalOutput")

    with tile.TileContext(nc) as tc:
        with (
            tc.tile_pool(name="dram", bufs=1, space="DRAM") as dp,
            tc.tile_pool(name="stat", bufs=1) as st,
            tc.tile_pool(name="tab", bufs=3) as tp,
            tc.tile_pool(name="tabp", bufs=2, space="PSUM") as tpp,
            tc.tile_pool(name="edge", bufs=5) as ep,
            tc.tile_pool(name="big", bufs=1) as bp,
            tc.tile_pool(name="edgep", bufs=2, space="PSUM") as epp,
        ):
            # ---- dram intermediates
            AC = [dp.tile([N_TOT + 1, 10], F32, tag=f"AC{l}", name=f"AC{l}") for l in range(2)]
            BD = [dp.tile([N_TOT + 1, 10], F32, tag=f"BD{l}", name=f"BD{l}") for l in range(2)]
            h1d = dp.tile([N_TOT, 5], F32, tag="h1d")
            aggcell = [dp.tile([NCELL_TOT + 8, 5], F32, tag=f"agc{l}", name=f"agc{l}") for l in range(2)]
            aggin = [dp.tile([SP, 5], F32, tag=f"agi{l}", name=f"agi{l}") for l in range(2)]
            aggout = [dp.tile([N_TOT, 5], F32, tag=f"ago{l}", name=f"ago{l}",
                               addr_space="Shared") for l in range(2)]
            CT1 = dp.tile([NCELL512, 20], F32, tag="CT1", name="CT1")
            CT2 = dp.tile([NCELL512, 10], F32, tag="CT2", name="CT2")

            # ---- static sbuf tiles
            def load(dr, shape, dtype=F32):
                t = st.tile(shape, dtype, name=dr.name + "_s")
                nc.sync.dma_start(t[:], dr[:])
                return t

            lhsT1_s = load(lhsT1, [10, 125])
            bias1_s = load(bias1, [1, 125])
            lhsT2_s = load(lhsT2, [25, 100])
            bias2_s = load(bias2, [1, 100])
            lhsTo_s = load(lhsTo, [25, 10])
            biaso_s = load(biaso, [1, 10])
            wpat_s = load(wpat, [128, 20])
            lseg_s = load(lseg, [128, 8])
            ones_s = load(onesd, [1, 512])
            lhsTc_s = load(lhsTc, [8, 80])
            biasc_s = load(biasc, [1, 80])
            lexp_s = load(lexp, [CPC, 128])
            wmix2_s = load(wmix2, [128, 50])

            # ---- poison / zero rows
            zp = st.tile([1, 10], F32)
            nc.vector.memset(zp[:], 0.0)
            pz = st.tile([1, 10], F32)
            nc.vector.memset(pz[:, 0:5], 0.0)
            nc.vector.memset(pz[:, 5:10], POISON)
            z40 = st.tile([1, 40], F32)
            nc.vector.memset(z40[:], 0.0)
            for l in range(2):
                nc.sync.dma_start(AC[l][N_TOT : N_TOT + 1, :], zp[:])
                nc.sync.dma_start(BD[l][N_TOT : N_TOT + 1, :], pz[:])
                zv = aggcell[l][NCELL_TOT : NCELL_TOT + 8, :].rearrange(
                    "r c -> (r c)").unsqueeze(0)
                nc.sync.dma_start(zv, z40[:])

            # table AP helper: rows (n'*5+g), channels [ch0, ch0+chn), n' range
            def tview(t, g, j0, w, ch0, chn):
                a = t[:N_TOT, :].rearrange("(ng five) ch -> ch ng five", five=5)
                return a[ch0 : ch0 + chn, j0 : j0 + w, g]

            def vview(t, width, g, j0, w, ch0, chn):
                a = t[:, :].rearrange("(ng five) ch -> ch ng five", five=5)
                return a[ch0 : ch0 + chn, j0 : j0 + w, g]

            # ---- table pass
            def table_pass(layer, rhs_src, rhs_width, lhs_s, bias_s, m_per_g,
                           evac):
                # rhs_src: callable(j0, w) -> AP [rhs_width, w] in SBUF
                for j in range(NGC):
                    j0 = j * 512
                    w = min(512, NG - j0)
                    ps = tpp.tile([128, 512], F32, space="PSUM", tag="tab")
                    m_tot = 5 * m_per_g
                    # bias prefill: out[m, n] = bias[m] * ones[n]
                    nc.tensor.matmul(ps[:m_tot, :w],
                                     lhsT=bias_s[:, :m_tot],
                                     rhs=ones_s[:, :w], start=True, stop=False)
                    nc.tensor.matmul(ps[:m_tot, :w], lhsT=lhs_s[:],
                                     rhs=rhs_src(j0, w), start=False, stop=True)
                    sb = tp.tile([128, 512], F32, tag="tabsb")
                    nc.vector.tensor_copy(sb[:m_tot, :w], ps[:m_tot, :w])
                    evac(j0, w, sb)

            # layer-1 tables: stream xT per chunk
            def rhs1(j0, w):
                xt = tp.tile([10, 512], F32, tag="xt")
                nc.sync.dma_start(xt[:, :w], xT[:, j0 : j0 + w])
                return xt[:, :w]

            def evac1(j0, w, ps):
                for g in range(5):
                    m0 = g * 25
                    nc.sync.dma_start(tview(BD[0], g, j0, w, 0, 10), ps[m0 + 10 : m0 + 20, :w])
                    nc.sync.dma_start(vview(h1d, 5, g, j0, w, 0, 5), ps[m0 + 20 : m0 + 25, :w])

            table_pass(0, rhs1, 10, lhsT1_s, bias1_s, 25, evac1)

            # ---- cell pass: CT1 = [A1 C1 | U2A U2C] per cell (cell order)
            for i in range(NCIT):
                ps = tpp.tile([128, 512], F32, space="PSUM", tag="tab",
                              name="ps_cell")
                nc.tensor.matmul(ps[:, :80], lhsT=ones_s[:, :128],
                                 rhs=biasc_s[:], start=True, stop=False)
                xt = tp.tile([8, 128], F32, tag="xcl")
                nc.sync.dma_start(xt[:], xcTP[:, i * 128 : (i + 1) * 128])
                nc.tensor.matmul(ps[:, :80], lhsT=xt[:], rhs=lhsTc_s[:],
                                 start=False, stop=True)
                sb = tp.tile([128, 80], F32, tag="clsb")
                nc.vector.tensor_copy(sb[:], ps[:, :80])
                base = i * 512
                dv = CT1[base : base + 512, 0:20].rearrange(
                    "(b pp) ch -> pp b ch", b=4)
                nc.sync.dma_start(dv, sb[:].rearrange(
                    "pp (b ch) -> pp b ch", ch=20))

            # ---- edge stream
            def edge_stream(layer):
                bdd, agc = BD[layer], aggcell[layer]
                wf0 = 10 * layer
                ctsrc = CT1 if layer == 0 else CT2
                for t in range(CHUNKS):
                    c0 = t * NCOLS
                    sidx = ep.tile([128, NCOLS], I32, tag="sidx")
                    nc.sync.dma_start(sidx[:], srcg[:, c0 : c0 + NCOLS])
                    esl = ep.tile([128, NCOLS], F32, tag="esl")
                    nc.sync.dma_start(esl[:], eg[:, c0 : c0 + NCOLS])

                    # dst side: celltab read (affine) + PE expansion
                    ctt = ep.tile([CPC, NCOLS * 10], F32, tag="ctt")
                    cb = c0 * CPC
                    dvv = ctsrc[cb : cb + NCOLS * CPC, 0:10].rearrange(
                        "(col cc) ch -> cc col ch", cc=CPC)
                    nc.sync.dma_start(ctt[:].rearrange(
                        "cc (col ch) -> cc col ch", ch=10), dvv)
                    acexp = epp.tile([128, NCOLS * 10], F32, space="PSUM",
                                     tag="acexp")
                    nc.tensor.matmul(acexp[:], lhsT=lexp_s[:], rhs=ctt[:],
                                     start=True, stop=True)

                    bdg = ep.tile([128, NCOLS * 10], F32, tag="bdg")
                    for col in range(NCOLS):
                        nc.gpsimd.indirect_dma_start(
                            out=bdg[:, col * 10 : col * 10 + 10],
                            out_offset=None, in_=bdd[:],
                            in_offset=bass.IndirectOffsetOnAxis(
                                ap=sidx[:, col : col + 1], axis=0))

                    # 3D views [128, NCOLS, 5]
                    def chv(tile_, ch0):
                        return tile_[:].rearrange("p (col ch) -> p col ch", ch=10)[
                            :, :, ch0 : ch0 + 5]

                    def v3(tile_):
                        return tile_[:].rearrange("p (col ch) -> p col ch", ch=5)

                    ebc = esl[:].unsqueeze(2).broadcast_to([128, NCOLS, 5])
                    wfb = wpat_s[:, wf0 : wf0 + 5].unsqueeze(1).broadcast_to(
                        [128, NCOLS, 5])
                    wsb = wpat_s[:, wf0 + 5 : wf0 + 10].unsqueeze(1).broadcast_to(
                        [128, NCOLS, 5])

                    ew = ep.tile([128, NCOLS * 5], F32, tag="ew")
                    tf = ep.tile([128, NCOLS * 5], F32, tag="tf")
                    nc.vector.tensor_tensor(out=v3(ew), in0=ebc, in1=wfb,
                                            op=mybir.AluOpType.mult)
                    nc.vector.tensor_tensor(out=v3(tf), in0=chv(acexp, 0),
                                            in1=chv(bdg, 0), op=mybir.AluOpType.add)
                    nc.vector.tensor_tensor(out=tf[:], in0=tf[:], in1=ew[:],
                                            op=mybir.AluOpType.add)
                    nc.vector.tensor_scalar_max(tf[:], tf[:], -80.0)
                    qf = ep.tile([128, NCOLS * 5], F32, tag="qf")
                    nc.scalar.activation(qf[:], tf[:], AF.Exp, scale=-1.0)
                    nc.vector.tensor_scalar_add(qf[:], qf[:], 1.0)
                    gf = ep.tile([128, NCOLS * 5], F32, tag="gf")
                    nc.vector.reciprocal_approx_fast(gf[:], qf[:])

                    ts = ep.tile([128, NCOLS * 5], F32, tag="ts")
                    nc.vector.tensor_tensor(out=v3(ew), in0=ebc, in1=wsb,
                                            op=mybir.AluOpType.mult)
                    nc.vector.tensor_tensor(out=v3(ts), in0=chv(acexp, 5),
                                            in1=chv(bdg, 5), op=mybir.AluOpType.add)
                    nc.vector.tensor_tensor(out=ts[:], in0=ts[:], in1=ew[:],
                                            op=mybir.AluOpType.add)
                    tsc = ep.tile([128, NCOLS * 5], F32, tag="tsc")
                    nc.vector.tensor_scalar_min(tsc[:], ts[:], 30.0)
                    rs = ep.tile([128, NCOLS * 5], F32, tag="rs")
                    nc.scalar.activation(rs[:], tsc[:], AF.Exp)
                    gs = ep.tile([128, NCOLS * 5], F32, tag="gs")
                    nc.scalar.activation(gs[:], rs[:], AF.Ln, bias=1.0)
                    nc.vector.tensor_tensor(out=gs[:], in0=gs[:], in1=ts[:],
                                            op=mybir.AluOpType.max)

                    msg = ep.tile([128, NCOLS * 5], F32, tag="msg")
                    nc.vector.tensor_tensor(out=msg[:], in0=gf[:], in1=gs[:],
                                            op=mybir.AluOpType.mult)

                    seg = epp.tile([8, NCOLS * 5], F32, space="PSUM", tag="seg")
                    nc.tensor.matmul(seg[:], lhsT=lseg_s[:], rhs=msg[:],
                                     start=True, stop=True)
                    if debug and layer == 0 and t == 0:
                        acexp_sb = ep.tile([128, NCOLS * 10], F32, tag="dbgsb")
                        nc.vector.tensor_copy(acexp_sb[:], acexp[:])
                        nc.sync.dma_start(dbg_ch["acexp"][:], acexp_sb[:])
                        nc.sync.dma_start(dbg_ch["bdg"][:], bdg[:])
                        for nm, tl in [("ew", ew), ("tf", tf), ("ts", ts),
                                       ("gf", gf), ("gs", gs), ("msg", msg)]:
                            nc.sync.dma_start(dbg_ch2[nm][:], tl[:])
                    segs = ep.tile([8, NCOLS * 5], F32, tag="segs")
                    nc.vector.tensor_copy(segs[:], seg[:])
                    # cell id = (c0+col)*8+cell
                    dv = agc[c0 * CPC * 1 : (c0 + NCOLS) * CPC, :].rearrange(
                        "(col cell) ch -> cell col ch", cell=CPC)
                    nc.sync.dma_start(dv, segs[:].rearrange(
                        "cell (col ch) -> cell col ch", ch=5))

            edge_stream(0)

            # ---- compact + collective (layer 1)
            def compact(layer):
                fct = bp.tile([128, K2], I32, tag="fct")
                nc.sync.dma_start(fct[:], fcd[:])
                mk = bp.tile([128, K2 * CPC * 5], F32, tag="mk")
                nc.sync.dma_start(mk[:], cmask[:])
                gc = bp.tile([128, K2 * CPC * 5], F32, tag="gc")
                for m in range(K2):
                    nc.gpsimd.indirect_dma_start(
                        out=gc[:, m * 40 : (m + 1) * 40],
                        out_offset=None, in_=aggcell[layer][:],
                        in_offset=bass.IndirectOffsetOnAxis(
                            ap=fct[:, m : m + 1], axis=0))
                nc.vector.tensor_tensor(out=gc[:], in0=gc[:], in1=mk[:],
                                        op=mybir.AluOpType.mult)
                def cv(t, a, b):
                    return t[:].rearrange("p (k cc ch) -> p k cc ch", cc=CPC, ch=5)[
                        :, :, a:b, :]
                h1_ = bp.tile([128, K2 * 4 * 5], F32, tag="ch1")
                nc.vector.tensor_tensor(out=h1_[:].rearrange(
                    "p (k cc ch) -> p k cc ch", cc=4, ch=5),
                    in0=cv(gc, 0, 4), in1=cv(gc, 4, 8), op=mybir.AluOpType.add)
                h2_ = bp.tile([128, K2 * 2 * 5], F32, tag="ch2")
                nc.vector.tensor_tensor(out=h2_[:].rearrange(
                    "p (k cc ch) -> p k cc ch", cc=2, ch=5),
                    in0=h1_[:].rearrange("p (k cc ch) -> p k cc ch", cc=4, ch=5)[:, :, 0:2, :],
                    in1=h1_[:].rearrange("p (k cc ch) -> p k cc ch", cc=4, ch=5)[:, :, 2:4, :],
                    op=mybir.AluOpType.add)
                acc = bp.tile([128, K2 * 5], F32, tag="cacc")
                nc.vector.tensor_tensor(out=acc[:].rearrange(
                    "p (k one ch) -> p k one ch", one=1, ch=5),
                    in0=h2_[:].rearrange("p (k cc ch) -> p k cc ch", cc=2, ch=5)[:, :, 0:1, :],
                    in1=h2_[:].rearrange("p (k cc ch) -> p k cc ch", cc=2, ch=5)[:, :, 1:2, :],
                    op=mybir.AluOpType.add)
                nc.sync.dma_start(
                    aggin[layer][:].rearrange("(pp k) ch -> pp (k ch)", k=K2),
                    acc[:])
                nc.gpsimd.collective_compute(
                    "AllGather", mybir.AluOpType.bypass,
                    replica_groups=[list(range(N_CORES))],
                    ins=[aggin[layer][:]], outs=[aggout[layer][:]])

            compact(0)

            # ---- layer-2 tables: h2 = h1 + agg1
            def h_view(t, j0, w):
                # [N_TOT, 5] -> [25, w]: partition (g*5+ch), free n'
                return t[:, :].rearrange("(ng five) ch -> (five ch) ng", five=5)[
                    :, j0 : j0 + w]

            def rhs2(j0, w):
                ht = tp.tile([25, 512], F32, tag="ht")
                at = tp.tile([25, 512], F32, tag="at")
                nc.sync.dma_start(ht[:, :w], h_view(h1d, j0, w))
                nc.sync.dma_start(at[:, :w], h_view(aggout[0], j0, w))
                nc.vector.tensor_tensor(out=ht[:, :w], in0=ht[:, :w], in1=at[:, :w],
                                        op=mybir.AluOpType.add)
                return ht[:, :w]

            def evac2(j0, w, ps):
                for g in range(5):
                    m0 = g * 20
                    nc.sync.dma_start(tview(BD[1], g, j0, w, 0, 10), ps[m0 + 10 : m0 + 20, :w])

            table_pass(1, rhs2, 25, lhsT2_s, bias2_s, 20, evac2)

            # ---- celltab2: CT2 = CT1[:,10:20] + gather(aggout[0])@w2
            HALF = NCALL // 2
            for half in range(2):
                m0 = half * HALF
                mN = NCALL - m0 if half == 1 else HALF
                cidx = bp.tile([128, mN], I32, tag="cidx", name="cidx")
                nc.sync.dma_start(cidx[:], aggidx[:, m0 : m0 + mN])
                gag = bp.tile([128, mN * 5], F32, tag="gag", name="gag")
                for m in range(mN):
                    nc.gpsimd.indirect_dma_start(
                        out=gag[:, m * 5 : (m + 1) * 5],
                        out_offset=None, in_=aggout[0][:],
                        in_offset=bass.IndirectOffsetOnAxis(
                            ap=cidx[:, m : m + 1], axis=0))
                u2 = bp.tile([128, mN * 10], F32, tag="u2c", name="u2c")
                dvr = CT1[m0 * 128 : (m0 + mN) * 128, 10:20].rearrange(
                    "(x pp) ch -> pp x ch", pp=128)
                nc.sync.dma_start(u2[:].rearrange(
                    "pp (x ch) -> pp x ch", ch=10), dvr)
                tmp = bp.tile([128, mN * 10], F32, tag="c2t", name="c2t")
                for c in range(5):
                    gb = gag[:].rearrange("pp (x ch) -> pp x ch", ch=5)[
                        :, :, c : c + 1].broadcast_to([128, mN, 10])
                    wb = wmix2_s[:, c * 10 : c * 10 + 10].unsqueeze(
                        1).broadcast_to([128, mN, 10])
                    nc.vector.tensor_tensor(
                        out=tmp[:].rearrange("pp (x ch) -> pp x ch", ch=10),
                        in0=gb, in1=wb, op=mybir.AluOpType.mult)
                    nc.vector.tensor_tensor(out=u2[:], in0=u2[:], in1=tmp[:],
                                            op=mybir.AluOpType.add)
                dvw = CT2[m0 * 128 : (m0 + mN) * 128, 0:10].rearrange(
                    "(x pp) ch -> pp x ch", pp=128)
                nc.sync.dma_start(dvw, u2[:].rearrange(
                    "pp (x ch) -> pp x ch", ch=10))

            edge_stream(1)
            compact(1)

            if debug:
                nc.sync.dma_start(dbg_h1[:], h1d[:])
                nc.sync.dma_start(dbg_ac[:], AC[0][:])
                nc.sync.dma_start(dbg_bd[:], BD[0][:])
                nc.sync.dma_start(dbg_agc[:], aggcell[0][: NCELL_TOT + 1, :])
                nc.sync.dma_start(dbg_ago[:], aggout[0][:])
                nc.sync.dma_start(dbg_ac2[:], AC[1][:])
                nc.sync.dma_start(dbg_bd2[:], BD[1][:])
                nc.sync.dma_start(dbg_ago2[:], aggout[1][:])

            # ---- final: h3 = h1 + agg1 + agg2; out = h3 @ lin2 + b
            for j in range(NGC):
                j0 = j * 512
                w = min(512, NG - j0)
                ht = tp.tile([25, 512], F32, tag="fht")
                at = tp.tile([25, 512], F32, tag="fat")
                nc.sync.dma_start(ht[:, :w], h_view(h1d, j0, w))
                nc.sync.dma_start(at[:, :w], h_view(aggout[0], j0, w))
                nc.vector.tensor_tensor(out=ht[:, :w], in0=ht[:, :w], in1=at[:, :w],
                                        op=mybir.AluOpType.add)
                nc.sync.dma_start(at[:, :w], h_view(aggout[1], j0, w))
                nc.vector.tensor_tensor(out=ht[:, :w], in0=ht[:, :w], in1=at[:, :w],
                                        op=mybir.AluOpType.add)
                ps = tpp.tile([128, 512], F32, space="PSUM", tag="fps")
                nc.tensor.matmul(ps[:10, :w], lhsT=biaso_s[:],
                                 rhs=ones_s[:, :w], start=True, stop=False)
                nc.tensor.matmul(ps[:10, :w], lhsT=lhsTo_s[:], rhs=ht[:, :w],
                                 start=False, stop=True)
                osb = tp.tile([10, 512], F32, tag="osb")
                nc.vector.tensor_copy(osb[:, :w], ps[:10, :w])
                for g in range(5):
                    nc.sync.dma_start(
                        out[:, :].rearrange("(ng five) ch -> ch ng five", five=5)[
                            :, j0 : j0 + w, g],
                        osb[g * 2 : g * 2 + 2, :w])

    nc.finalize()
    return nc


# ----------------------------------------------------------------------------
# weights packing
# ----------------------------------------------------------------------------
def _host_arrays(p, inputs, n_nodes):
    N_TOT, NG, SP = p.N_TOT, p.NG, p.SP
    x = np.asarray(inputs["x"], np.float32)
    lin1_w = np.asarray(inputs["lin1_w"], np.float32)
    lin1_b = np.asarray(inputs["lin1_b"], np.float32)
    lin2_w = np.asarray(inputs["lin2_w"], np.float32)
    lin2_b = np.asarray(inputs["lin2_b"], np.float32)
    wf1 = np.asarray(inputs["conv1_wf"], np.float32)
    bf1 = np.asarray(inputs["conv1_bf"], np.float32)
    ws1 = np.asarray(inputs["conv1_ws"], np.float32)
    bs1 = np.asarray(inputs["conv1_bs"], np.float32)
    wf2 = np.asarray(inputs["conv2_wf"], np.float32)
    bf2 = np.asarray(inputs["conv2_bf"], np.float32)
    ws2 = np.asarray(inputs["conv2_ws"], np.float32)
    bs2 = np.asarray(inputs["conv2_bs"], np.float32)

    rel2orig = np.full(N_TOT, -1, dtype=np.int64)
    rel2orig[p.orig2rel] = np.arange(n_nodes)
    valid = rel2orig >= 0
    x_rel = np.zeros((N_TOT, 2), np.float32)
    x_rel[valid] = x[rel2orig[valid]]

    xT = _pack_T(x_rel, p, 2)  # [10, NG]

    # layer-1 combined projections: [A'|C'|B|D|h] (25 ch per group)
    W1 = np.concatenate([
        lin1_w @ wf1[0:CH], lin1_w @ ws1[0:CH],
        lin1_w @ wf1[CH : 2 * CH], lin1_w @ ws1[CH : 2 * CH],
        lin1_w,
    ], axis=1)  # [2, 25]
    b1 = np.concatenate([
        lin1_b @ wf1[0:CH] + bf1, lin1_b @ ws1[0:CH] + bs1,
        lin1_b @ wf1[CH : 2 * CH], lin1_b @ ws1[CH : 2 * CH],
        lin1_b,
    ])  # [25]
    lhsT1 = np.zeros((10, 125), np.float32)
    bias1 = np.zeros((1, 125), np.float32)
    for g in range(5):
        lhsT1[g * 2 : g * 2 + 2, g * 25 : (g + 1) * 25] = W1
        bias1[0, g * 25 : (g + 1) * 25] = b1

    W2 = np.concatenate([wf2[0:CH], ws2[0:CH], wf2[CH : 2 * CH], ws2[CH : 2 * CH]],
                        axis=1)  # [5, 20]
    b2 = np.concatenate([bf2, bs2, np.zeros(10, np.float32)])
    lhsT2 = np.zeros((25, 100), np.float32)
    bias2 = np.zeros((1, 100), np.float32)
    for g in range(5):
        lhsT2[g * 5 : (g + 1) * 5, g * 20 : (g + 1) * 20] = W2
        bias2[0, g * 20 : (g + 1) * 20] = b2

    lhsTo = np.zeros((25, 10), np.float32)
    biaso = np.zeros((1, 10), np.float32)
    for g in range(5):
        lhsTo[g * 5 : (g + 1) * 5, g * 2 : (g + 1) * 2] = lin2_w
        biaso[0, g * 2 : (g + 1) * 2] = lin2_b

    wpat = np.zeros((128, 20), np.float32)
    wpat[:, 0:5] = wf1[2 * CH]
    wpat[:, 5:10] = ws1[2 * CH]
    wpat[:, 10:15] = wf2[2 * CH]
    wpat[:, 15:20] = ws2[2 * CH]

    lseg = np.zeros((128, 8), np.float32)
    for cc in range(CPC):
        lseg[cc * CELL : (cc + 1) * CELL, cc] = 1.0
    onesd = np.ones((1, 512), np.float32)
    lexp = np.zeros((CPC, 128), np.float32)
    for r in range(128):
        lexp[r // CELL, r] = 1.0
    # cell-pass weights: out ch = [A1(5) C1(5) U2A(5) U2C(5)] = 20 per block
    BLK = 4
    Wc = np.concatenate([lin1_w @ wf1[0:CH], lin1_w @ ws1[0:CH],
                         lin1_w @ wf2[0:CH], lin1_w @ ws2[0:CH]], axis=1)
    bcv = np.concatenate([lin1_b @ wf1[0:CH] + bf1, lin1_b @ ws1[0:CH] + bs1,
                          lin1_b @ wf2[0:CH] + bf2, lin1_b @ ws2[0:CH] + bs2])
    lhsTc = np.zeros((2 * BLK, BLK * 20), np.float32)
    biasc = np.zeros((1, BLK * 20), np.float32)
    for b in range(BLK):
        for xc in range(2):
            lhsTc[xc * BLK + b, b * 20 : (b + 1) * 20] = Wc[xc]
        biasc[0, b * 20 : (b + 1) * 20] = bcv
    # agg->AC2 transform rows: wmix2[:, c*10 : c*10+10] = [wf2[c,:] | ws2[c,:]]
    wmix2 = np.zeros((128, 50), np.float32)
    for c2 in range(CH):
        wmix2[:, c2 * 10 : c2 * 10 + 5] = wf2[c2]
        wmix2[:, c2 * 10 + 5 : c2 * 10 + 10] = ws2[c2]

    shared = {
        "xT": xT, "lhsT1": lhsT1, "bias1": bias1, "lhsT2": lhsT2, "bias2": bias2,
        "lhsTo": lhsTo, "biaso": biaso, "wpat": wpat, "lseg": lseg,
        "onesd": onesd, "lhsTc": lhsTc, "biasc": biasc, "lexp": lexp,
        "wmix2": wmix2,
    }
    in_maps = []
    for k in range(N_CORES):
        c = p.core[k]
        m = dict(shared)
        m["srcg"] = c["src_g"]
        m["eg"] = c["e_g"]
        m["fcd"] = c["fc"]
        m["cmask"] = c["cmask"]
        m["aggidx"] = c["aggidx"]
        # xcell pack [8, NCELL512/4]: row xc*4+b, col i*128+pp ;
        # cell = i*512 + b*128 + pp
        cl = c["cell_lin"]
        xcell = np.zeros((p.NCELL512, 2), np.float32)
        real = cl < p.N_TOT
        xcell[real] = x_rel[cl[real]]
        ncit = p.NCELL512 // 512
        xcv = xcell.reshape(ncit, 4, 128, 2)
        m["xcTP"] = np.ascontiguousarray(
            np.transpose(xcv, (3, 1, 0, 2)).reshape(8, -1))
        in_maps.append(m)
    return in_maps, rel2orig


# ----------------------------------------------------------------------------
# entry point
# ----------------------------------------------------------------------------
_CACHE = {}


def kernel_impl(inputs, n_nodes):
    ei = np.asarray(inputs["edge_index"])
    ea = np.asarray(inputs["edge_attr"])
    key = (ei.shape[1], n_nodes)
    p = _prepare(ei, ea, n_nodes)
    if key not in _CACHE:
        _CACHE[key] = _build(p)
    nc = _CACHE[key]
    in_maps, rel2orig = _host_arrays(p, inputs, n_nodes)
    res = run_bass_kernel_spmd(nc, in_maps, core_ids=list(range(N_CORES)))
    out_rel = res.results[0]["out"]
    return np.ascontiguousarray(out_rel[p.orig2rel]).astype(np.float32)


def kernel(**inputs):
    return kernel_impl(inputs, 100000)



# revision 13
# speedup vs baseline: 1.0131x; 1.0131x over previous
"""Trainium2 Bass kernel for the 2-layer CGConv GNN (nn_Net_90881507983728).

Self-contained: host-side prep (sort/shard/pack) + Bass program + SPMD run.

Strategy (8 cores):
  - Edges sorted by dst; nodes split into 8 contiguous ranges balanced by
    padded-degree; per core, nodes packed into 128-slot columns of 8 fixed
    16-row cells (degree padded to multiple of 16).
  - Per conv layer: PE computes node tables [A'|C'] (dst projections + bias)
    and [B|D] (src projections) for all nodes; per edge chunk: dst-side values
    expanded from per-cell gathers by a static PE matmul; src-side values
    fetched by batched indirect DMA gather; DVE/ACT compute
    sigmoid(.)*softplus(.); a static block-ones PE matmul produces per-cell
    sums; per-node sums assembled by a small gather+add; AllGather exchanges
    per-core aggregate slices.
  - Pad/dummy slots have src pointing at a poisoned table row (-30000) so
    their messages are exactly 0.
"""
import numpy as np

import concourse.bass as bass
import concourse.bacc as bacc
import concourse.mybir as mybir
import concourse.tile as tile
from concourse.bass_utils import run_bass_kernel_spmd

N_CORES = 8
CELL = 16
CPC = 8            # cells per column
NCOLS = 51         # columns per chunk (51*10 = 510 <= 512 PSUM free)
CH = 5
F32 = mybir.dt.float32
I32 = mybir.dt.int32
POISON = -30000.0

AF = mybir.ActivationFunctionType


# ----------------------------------------------------------------------------
# host prep
# ----------------------------------------------------------------------------
class P:
    pass


def _prepare(edge_index, edge_attr, n_nodes):
    p = P()
    src = edge_index[0].astype(np.int64)
    dst = edge_index[1].astype(np.int64)
    e = edge_attr.reshape(-1).astype(np.float32)

    deg = np.bincount(dst, minlength=n_nodes).astype(np.int64)
    assert deg.max() <= 128, f"max degree {deg.max()} > 128 unsupported"
    ncells_all = (deg + CELL - 1) // CELL          # 0 cells for deg-0 nodes
    pad_deg = ncells_all * CELL

    cum = np.cumsum(pad_deg)
    total = cum[-1]
    bounds = [0]
    for k in range(1, N_CORES):
        bounds.append(int(np.searchsorted(cum, total * k / N_CORES)))
    bounds.append(n_nodes)

    order = np.argsort(dst, kind="stable")
    src_s = src[order]
    e_s = e[order]
    starts = np.zeros(n_nodes + 1, dtype=np.int64)
    np.cumsum(deg, out=starts[1:])

    # dense packing: each core concatenates its nodes' cells in node order
    # (cells may straddle column boundaries; compact only needs consecutive
    # cell ids per node)
    max_cols = 0
    max_nodes = 0
    for k in range(N_CORES):
        n0, n1 = bounds[k], bounds[k + 1]
        ncell_k = int(ncells_all[n0:n1].sum())
        max_cols = max(max_cols, (ncell_k + CPC - 1) // CPC)
        max_nodes = max(max_nodes, n1 - n0)

    CHUNKS = (max_cols + NCOLS - 1) // NCOLS
    C_pad = CHUNKS * NCOLS
    SP = ((max_nodes + 639) // 640) * 640   # multiple of 128 and 5
    K2 = SP // 128
    N_TOT = SP * N_CORES
    NG = N_TOT // 5
    ZROW = N_TOT
    NCELL_TOT = C_pad * CPC
    ZCELL = NCELL_TOT

    p.CHUNKS, p.C_pad, p.SP, p.K2 = CHUNKS, C_pad, SP, K2
    p.N_TOT, p.NG, p.ZROW, p.NCELL_TOT, p.ZCELL = N_TOT, NG, ZROW, NCELL_TOT, ZCELL
    p.NCELL512 = ((NCELL_TOT + 511) // 512) * 512

    orig2rel = np.zeros(n_nodes, dtype=np.int64)
    for k in range(N_CORES):
        n0, n1 = bounds[k], bounds[k + 1]
        orig2rel[n0:n1] = k * SP + np.arange(n1 - n0)
    p.core = []
    for k in range(N_CORES):
        n0, n1 = bounds[k], bounds[k + 1]
        nodes = np.arange(n0, n1)
        nn = len(nodes)
        lid = np.arange(nn)
        ncl = ncells_all[nodes]                       # cells per node
        cellstart = np.zeros(nn + 1, dtype=np.int64)
        np.cumsum(ncl, out=cellstart[1:])
        ncell_k = int(cellstart[-1])

        # per-edge slot fill (slots linear: cell*CELL + s)
        d_k = deg[nodes]
        tot_e = int(d_k.sum())
        estart = np.zeros(nn + 1, dtype=np.int64)
        np.cumsum(d_k, out=estart[1:])
        ln = np.repeat(lid, d_k)
        r = np.arange(tot_e) - np.repeat(estart[:-1], d_k)
        gpos = np.repeat(starts[nodes], d_k) + r
        slot = cellstart[ln] * CELL + r

        sidx_lin = np.full(NCELL_TOT * CELL, -1, dtype=np.int64)
        e_lin = np.zeros(NCELL_TOT * CELL, dtype=np.float32)
        sidx_lin[slot] = src_s[gpos]
        e_lin[slot] = e_s[gpos]
        # [C_pad*CPC*CELL] -> [128, C_pad]: col = slot//128, row = slot%128
        src_g = np.ascontiguousarray(sidx_lin.reshape(C_pad, 128).T)
        e_g = np.ascontiguousarray(e_lin.reshape(C_pad, 128).T)

        cn_lin = np.full(NCELL_TOT, ZROW, dtype=np.int64)
        cn_lin[:ncell_k] = k * SP + np.repeat(lid, ncl)
        cellnode_g = np.ascontiguousarray(
            cn_lin.reshape(C_pad, CPC).T.astype(np.int32))

        fc_n = np.full(SP, ZCELL, dtype=np.int64)
        fc_n[:nn] = np.where(ncl > 0, cellstart[:-1], ZCELL)
        ncl_full = np.zeros(SP, dtype=np.int64)
        ncl_full[:nn] = ncl

        c = {}
        mask = src_g >= 0
        sg2 = np.full(src_g.shape, ZROW, dtype=np.int32)
        sg2[mask] = orig2rel[src_g[mask]]
        c["src_g"] = sg2
        c["e_g"] = e_g
        c["cellnode_g"] = cellnode_g
        # cell-linear node list + agg-gather idx (celltab2)
        cl512 = np.concatenate([cn_lin, np.full(p.NCELL512 - NCELL_TOT, ZROW,
                                                np.int64)])
        c["cell_lin"] = cl512
        safe = np.where(cl512 >= N_TOT, 0, cl512).astype(np.int32)
        c["aggidx"] = np.ascontiguousarray(safe.reshape(-1, 128).T)
        c["fc"] = fc_n.reshape(128, K2).astype(np.int32)
        m = (np.arange(CPC)[None, :] < ncl_full[:, None]).astype(np.float32)
        c["cmask"] = np.ascontiguousarray(
            np.repeat(m, 5, axis=1).reshape(128, K2 * CPC * 5))
        p.core.append(c)
    p.orig2rel = orig2rel
    return p


def _pack_T(values, p, width):
    """[N_TOT, width] node-major -> [5*width, NG] with partition (grp*width+f)."""
    v = values[: p.N_TOT].reshape(p.NG, 5, width)
    return np.ascontiguousarray(
        np.transpose(v, (1, 2, 0)).reshape(5 * width, p.NG).astype(np.float32))


# ----------------------------------------------------------------------------
# bass program
# ----------------------------------------------------------------------------
def _build(p, debug=False):
    NG, SP, K2, N_TOT = p.NG, p.SP, p.K2, p.N_TOT
    CHUNKS, C_pad, NCELL_TOT = p.CHUNKS, p.C_pad, p.NCELL_TOT
    NGC = (NG + 511) // 512

    nc = bacc.Bacc("TRN2", target_bir_lowering=False, num_devices=N_CORES)

    # inputs
    xT = nc.dram_tensor("xT", [10, NG], F32, kind="ExternalInput")
    lhsT1 = nc.dram_tensor("lhsT1", [10, 125], F32, kind="ExternalInput")
    bias1 = nc.dram_tensor("bias1", [1, 125], F32, kind="ExternalInput")
    lhsT2 = nc.dram_tensor("lhsT2", [25, 100], F32, kind="ExternalInput")
    bias2 = nc.dram_tensor("bias2", [1, 100], F32, kind="ExternalInput")
    lhsTo = nc.dram_tensor("lhsTo", [25, 10], F32, kind="ExternalInput")
    biaso = nc.dram_tensor("biaso", [1, 10], F32, kind="ExternalInput")
    srcg = nc.dram_tensor("srcg", [128, C_pad], I32, kind="ExternalInput")
    eg = nc.dram_tensor("eg", [128, C_pad], F32, kind="ExternalInput")
    NCELL512 = p.NCELL512
    NCIT = NCELL512 // 512
    NCALL = NCELL512 // 128
    xcTP = nc.dram_tensor("xcTP", [8, NCELL512 // 4], F32, kind="ExternalInput")
    lhsTc = nc.dram_tensor("lhsTc", [8, 80], F32, kind="ExternalInput")
    biasc = nc.dram_tensor("biasc", [1, 80], F32, kind="ExternalInput")
    lexp = nc.dram_tensor("lexp", [CPC, 128], F32, kind="ExternalInput")
    wmix2 = nc.dram_tensor("wmix2", [128, 50], F32, kind="ExternalInput")
    aggidx = nc.dram_tensor("aggidx", [128, NCALL], I32, kind="ExternalInput")
    fcd = nc.dram_tensor("fcd", [128, K2], I32, kind="ExternalInput")
    cmask = nc.dram_tensor("cmask", [128, K2 * CPC * 5], F32, kind="ExternalInput")
    wpat = nc.dram_tensor("wpat", [128, 20], F32, kind="ExternalInput")
    lseg = nc.dram_tensor("lseg", [128, 8], F32, kind="ExternalInput")
    onesd = nc.dram_tensor("onesd", [1, 512], F32, kind="ExternalInput")
    out = nc.dram_tensor("out", [N_TOT, 2], F32, kind="ExternalOutput")
    if debug:
        dbg_ch = {nm: nc.dram_tensor(f"dbg_{nm}", [128, NCOLS * 10], F32, kind="ExternalOutput")
                  for nm in ["acexp", "bdg"]}
        dbg_ch2 = {nm: nc.dram_tensor(f"dbg_{nm}", [128, NCOLS * 5], F32, kind="ExternalOutput")
                   for nm in ["ew", "tf", "ts", "gf", "gs", "msg"]}
        dbg_h1 = nc.dram_tensor("dbg_h1", [N_TOT, 5], F32, kind="ExternalOutput")
        dbg_ac = nc.dram_tensor("dbg_ac", [N_TOT + 1, 10], F32, kind="ExternalOutput")
        dbg_bd = nc.dram_tensor("dbg_bd", [N_TOT + 1, 10], F32, kind="ExternalOutput")
        dbg_agc = nc.dram_tensor("dbg_agc", [NCELL_TOT + 1, 5], F32, kind="ExternalOutput")
        dbg_ago = nc.dram_tensor("dbg_ago", [N_TOT, 5], F32, kind="ExternalOutput")
        dbg_ac2 = nc.dram_tensor("dbg_ac2", [N_TOT + 1, 10], F32, kind="ExternalOutput")
        dbg_bd2 = nc.dram_tensor("dbg_bd2", [N_TOT + 1, 10], F32, kind="ExternalOutput")
        dbg_ago2 = nc.dram_tensor("dbg_ago2", [N_TOT, 5], F32, kind="ExternalOutput")

    with tile.TileContext(nc) as tc:
        with (
            tc.tile_pool(name="dram", bufs=1, space="DRAM") as dp,
            tc.tile_pool(name="stat", bufs=1) as st,
            tc.tile_pool(name="tab", bufs=3) as tp,
            tc.tile_pool(name="tabp", bufs=2, space="PSUM") as tpp,
            tc.tile_pool(name="edge", bufs=4) as ep,
            tc.tile_pool(name="big", bufs=1) as bp,
            tc.tile_pool(name="edgep", bufs=2, space="PSUM") as epp,
        ):
            # ---- dram intermediates
            AC = [dp.tile([N_TOT + 1, 10], F32, tag=f"AC{l}", name=f"AC{l}") for l in range(2)]
            BD = [dp.tile([N_TOT + 1, 10], F32, tag=f"BD{l}", name=f"BD{l}") for l in range(2)]
            h1d = dp.tile([N_TOT, 5], F32, tag="h1d")
            aggcell = [dp.tile([NCELL_TOT + 8, 5], F32, tag=f"agc{l}", name=f"agc{l}") for l in range(2)]
            aggin = [dp.tile([SP, 5], F32, tag=f"agi{l}", name=f"agi{l}") for l in range(2)]
            aggout = [dp.tile([N_TOT, 5], F32, tag=f"ago{l}", name=f"ago{l}",
                               addr_space="Shared") for l in range(2)]
            CT1 = dp.tile([NCELL512, 20], F32, tag="CT1", name="CT1")
            CT2 = dp.tile([NCELL512, 10], F32, tag="CT2", name="CT2")

            # ---- static sbuf tiles
            def load(dr, shape, dtype=F32):
                t = st.tile(shape, dtype, name=dr.name + "_s")
                nc.sync.dma_start(t[:], dr[:])
                return t

            lhsT1_s = load(lhsT1, [10, 125])
            bias1_s = load(bias1, [1, 125])
            lhsT2_s = load(lhsT2, [25, 100])
            bias2_s = load(bias2, [1, 100])
            lhsTo_s = load(lhsTo, [25, 10])
            biaso_s = load(biaso, [1, 10])
            wpat_s = load(wpat, [128, 20])
            lseg_s = load(lseg, [128, 8])
            ones_s = load(onesd, [1, 512])
            sidxall = load(srcg, [128, C_pad], I32)
            lhsTc_s = load(lhsTc, [8, 80])
            biasc_s = load(biasc, [1, 80])
            lexp_s = load(lexp, [CPC, 128])
            wmix2_s = load(wmix2, [128, 50])

            # ---- poison / zero rows
            zp = st.tile([1, 10], F32)
            nc.vector.memset(zp[:], 0.0)
            pz = st.tile([1, 10], F32)
            nc.vector.memset(pz[:, 0:5], 0.0)
            nc.vector.memset(pz[:, 5:10], POISON)
            z40 = st.tile([1, 40], F32)
            nc.vector.memset(z40[:], 0.0)
            for l in range(2):
                nc.sync.dma_start(AC[l][N_TOT : N_TOT + 1, :], zp[:])
                nc.sync.dma_start(BD[l][N_TOT : N_TOT + 1, :], pz[:])
                zv = aggcell[l][NCELL_TOT : NCELL_TOT + 8, :].rearrange(
                    "r c -> (r c)").unsqueeze(0)
                nc.sync.dma_start(zv, z40[:])

            # table AP helper: rows (n'*5+g), channels [ch0, ch0+chn), n' range
            def tview(t, g, j0, w, ch0, chn):
                a = t[:N_TOT, :].rearrange("(ng five) ch -> ch ng five", five=5)
                return a[ch0 : ch0 + chn, j0 : j0 + w, g]

            def vview(t, width, g, j0, w, ch0, chn):
                a = t[:, :].rearrange("(ng five) ch -> ch ng five", five=5)
                return a[ch0 : ch0 + chn, j0 : j0 + w, g]

            # ---- table pass
            def table_pass(layer, rhs_src, rhs_width, lhs_s, bias_s, m_per_g,
                           evac):
                # rhs_src: callable(j0, w) -> AP [rhs_width, w] in SBUF
                for j in range(NGC):
                    j0 = j * 512
                    w = min(512, NG - j0)
                    ps = tpp.tile([128, 512], F32, space="PSUM", tag="tab")
                    m_tot = 5 * m_per_g
                    # bias prefill: out[m, n] = bias[m] * ones[n]
                    nc.tensor.matmul(ps[:m_tot, :w],
                                     lhsT=bias_s[:, :m_tot],
                                     rhs=ones_s[:, :w], start=True, stop=False)
                    nc.tensor.matmul(ps[:m_tot, :w], lhsT=lhs_s[:],
                                     rhs=rhs_src(j0, w), start=False, stop=True)
                    sb = tp.tile([128, 512], F32, tag="tabsb")
                    nc.vector.tensor_copy(sb[:m_tot, :w], ps[:m_tot, :w])
                    evac(j0, w, sb)

            # layer-1 tables: stream xT per chunk
            def rhs1(j0, w):
                xt = tp.tile([10, 512], F32, tag="xt")
                nc.sync.dma_start(xt[:, :w], xT[:, j0 : j0 + w])
                return xt[:, :w]

            def evac1(j0, w, ps):
                for g in range(5):
                    m0 = g * 25
                    nc.sync.dma_start(tview(BD[0], g, j0, w, 0, 10), ps[m0 + 10 : m0 + 20, :w])
                    nc.sync.dma_start(vview(h1d, 5, g, j0, w, 0, 5), ps[m0 + 20 : m0 + 25, :w])

            table_pass(0, rhs1, 10, lhsT1_s, bias1_s, 25, evac1)

            # ---- cell pass: CT1 = [A1 C1 | U2A U2C] per cell (cell order)
            for i in range(NCIT):
                ps = tpp.tile([128, 512], F32, space="PSUM", tag="tab",
                              name="ps_cell")
                nc.tensor.matmul(ps[:, :80], lhsT=ones_s[:, :128],
                                 rhs=biasc_s[:], start=True, stop=False)
                xt = tp.tile([8, 128], F32, tag="xcl")
                nc.sync.dma_start(xt[:], xcTP[:, i * 128 : (i + 1) * 128])
                nc.tensor.matmul(ps[:, :80], lhsT=xt[:], rhs=lhsTc_s[:],
                                 start=False, stop=True)
                sb = tp.tile([128, 80], F32, tag="clsb")
                nc.vector.tensor_copy(sb[:], ps[:, :80])
                base = i * 512
                dv = CT1[base : base + 512, 0:20].rearrange(
                    "(b pp) ch -> pp b ch", b=4)
                nc.sync.dma_start(dv, sb[:].rearrange(
                    "pp (b ch) -> pp b ch", ch=20))

            # ---- edge stream
            def edge_stream(layer):
                bdd, agc = BD[layer], aggcell[layer]
                wf0 = 10 * layer
                ctsrc = CT1 if layer == 0 else CT2
                for t in range(CHUNKS):
                    c0 = t * NCOLS
                    esl = ep.tile([128, NCOLS], F32, tag="esl")
                    nc.sync.dma_start(esl[:], eg[:, c0 : c0 + NCOLS])

                    # dst side: celltab read (affine) + PE expansion
                    ctt = ep.tile([CPC, NCOLS * 10], F32, tag="ctt")
                    cb = c0 * CPC
                    dvv = ctsrc[cb : cb + NCOLS * CPC, 0:10].rearrange(
                        "(col cc) ch -> cc col ch", cc=CPC)
                    nc.sync.dma_start(ctt[:].rearrange(
                        "cc (col ch) -> cc col ch", ch=10), dvv)
                    acexp = epp.tile([128, NCOLS * 10], F32, space="PSUM",
                                     tag="acexp")
                    nc.tensor.matmul(acexp[:], lhsT=lexp_s[:], rhs=ctt[:],
                                     start=True, stop=True)

                    bdg = ep.tile([128, NCOLS * 10], F32, tag="bdg")
                    for col in range(NCOLS):
                        nc.gpsimd.indirect_dma_start(
                            out=bdg[:, col * 10 : col * 10 + 10],
                            out_offset=None, in_=bdd[:],
                            in_offset=bass.IndirectOffsetOnAxis(
                                ap=sidxall[:, c0 + col : c0 + col + 1],
                                axis=0))

                    # 3D views [128, NCOLS, 5]
                    def chv(tile_, ch0):
                        return tile_[:].rearrange("p (col ch) -> p col ch", ch=10)[
                            :, :, ch0 : ch0 + 5]

                    def v3(tile_):
                        return tile_[:].rearrange("p (col ch) -> p col ch", ch=5)

                    ebc = esl[:].unsqueeze(2).broadcast_to([128, NCOLS, 5])
                    wfb = wpat_s[:, wf0 : wf0 + 5].unsqueeze(1).broadcast_to(
                        [128, NCOLS, 5])
                    wsb = wpat_s[:, wf0 + 5 : wf0 + 10].unsqueeze(1).broadcast_to(
                        [128, NCOLS, 5])

                    ew = ep.tile([128, NCOLS * 5], F32, tag="ew")
                    tf = ep.tile([128, NCOLS * 5], F32, tag="tf")
                    nc.vector.tensor_tensor(out=v3(ew), in0=ebc, in1=wfb,
                                            op=mybir.AluOpType.mult)
                    nc.vector.tensor_tensor(out=v3(tf), in0=chv(acexp, 0),
                                            in1=chv(bdg, 0), op=mybir.AluOpType.add)
                    nc.vector.tensor_tensor(out=tf[:], in0=tf[:], in1=ew[:],
                                            op=mybir.AluOpType.add)
                    nc.vector.tensor_scalar_max(tf[:], tf[:], -80.0)
                    qf = ep.tile([128, NCOLS * 5], F32, tag="qf")
                    nc.scalar.activation(qf[:], tf[:], AF.Exp, scale=-1.0)
                    nc.vector.tensor_scalar_add(qf[:], qf[:], 1.0)
                    gf = ep.tile([128, NCOLS * 5], F32, tag="gf")
                    nc.vector.reciprocal_approx_fast(gf[:], qf[:])

                    ts = ep.tile([128, NCOLS * 5], F32, tag="ts")
                    nc.vector.tensor_tensor(out=v3(ew), in0=ebc, in1=wsb,
                                            op=mybir.AluOpType.mult)
                    nc.vector.tensor_tensor(out=v3(ts), in0=chv(acexp, 5),
                                            in1=chv(bdg, 5), op=mybir.AluOpType.add)
                    nc.vector.tensor_tensor(out=ts[:], in0=ts[:], in1=ew[:],
                                            op=mybir.AluOpType.add)
                    tsc = ep.tile([128, NCOLS * 5], F32, tag="tsc")
                    nc.vector.tensor_scalar_min(tsc[:], ts[:], 30.0)
                    rs = ep.tile([128, NCOLS * 5], F32, tag="rs")
                    nc.scalar.activation(rs[:], tsc[:], AF.Exp)
                    gs = ep.tile([128, NCOLS * 5], F32, tag="gs")
                    nc.scalar.activation(gs[:], rs[:], AF.Ln, bias=1.0)
                    nc.vector.tensor_tensor(out=gs[:], in0=gs[:], in1=ts[:],
                                            op=mybir.AluOpType.max)

                    msg = ep.tile([128, NCOLS * 5], F32, tag="msg")
                    nc.vector.tensor_tensor(out=msg[:], in0=gf[:], in1=gs[:],
                                            op=mybir.AluOpType.mult)

                    seg = epp.tile([8, NCOLS * 5], F32, space="PSUM", tag="seg")
                    nc.tensor.matmul(seg[:], lhsT=lseg_s[:], rhs=msg[:],
                                     start=True, stop=True)
                    if debug and layer == 0 and t == 0:
                        acexp_sb = ep.tile([128, NCOLS * 10], F32, tag="dbgsb")
                        nc.vector.tensor_copy(acexp_sb[:], acexp[:])
                        nc.sync.dma_start(dbg_ch["acexp"][:], acexp_sb[:])
                        nc.sync.dma_start(dbg_ch["bdg"][:], bdg[:])
                        for nm, tl in [("ew", ew), ("tf", tf), ("ts", ts),
                                       ("gf", gf), ("gs", gs), ("msg", msg)]:
                            nc.sync.dma_start(dbg_ch2[nm][:], tl[:])
                    segs = ep.tile([8, NCOLS * 5], F32, tag="segs")
                    nc.vector.tensor_copy(segs[:], seg[:])
                    # cell id = (c0+col)*8+cell
                    dv = agc[c0 * CPC * 1 : (c0 + NCOLS) * CPC, :].rearrange(
                        "(col cell) ch -> cell col ch", cell=CPC)
                    nc.sync.dma_start(dv, segs[:].rearrange(
                        "cell (col ch) -> cell col ch", ch=5))

            edge_stream(0)

            # ---- compact + collective (layer 1)
            def compact(layer):
                fct = bp.tile([128, K2], I32, tag="fct")
                nc.sync.dma_start(fct[:], fcd[:])
                mk = bp.tile([128, K2 * CPC * 5], F32, tag="mk")
                nc.sync.dma_start(mk[:], cmask[:])
                gc = bp.tile([128, K2 * CPC * 5], F32, tag="gc")
                for m in range(K2):
                    nc.gpsimd.indirect_dma_start(
                        out=gc[:, m * 40 : (m + 1) * 40],
                        out_offset=None, in_=aggcell[layer][:],
                        in_offset=bass.IndirectOffsetOnAxis(
                            ap=fct[:, m : m + 1], axis=0))
                nc.vector.tensor_tensor(out=gc[:], in0=gc[:], in1=mk[:],
                                        op=mybir.AluOpType.mult)
                def cv(t, a, b):
                    return t[:].rearrange("p (k cc ch) -> p k cc ch", cc=CPC, ch=5)[
                        :, :, a:b, :]
                h1_ = bp.tile([128, K2 * 4 * 5], F32, tag="ch1")
                nc.vector.tensor_tensor(out=h1_[:].rearrange(
                    "p (k cc ch) -> p k cc ch", cc=4, ch=5),
                    in0=cv(gc, 0, 4), in1=cv(gc, 4, 8), op=mybir.AluOpType.add)
                h2_ = bp.tile([128, K2 * 2 * 5], F32, tag="ch2")
                nc.vector.tensor_tensor(out=h2_[:].rearrange(
                    "p (k cc ch) -> p k cc ch", cc=2, ch=5),
                    in0=h1_[:].rearrange("p (k cc ch) -> p k cc ch", cc=4, ch=5)[:, :, 0:2, :],
                    in1=h1_[:].rearrange("p (k cc ch) -> p k cc ch", cc=4, ch=5)[:, :, 2:4, :],
                    op=mybir.AluOpType.add)
                acc = bp.tile([128, K2 * 5], F32, tag="cacc")
                nc.vector.tensor_tensor(out=acc[:].rearrange(
                    "p (k one ch) -> p k one ch", one=1, ch=5),
                    in0=h2_[:].rearrange("p (k cc ch) -> p k cc ch", cc=2, ch=5)[:, :, 0:1, :],
                    in1=h2_[:].rearrange("p (k cc ch) -> p k cc ch", cc=2, ch=5)[:, :, 1:2, :],
                    op=mybir.AluOpType.add)
                nc.sync.dma_start(
                    aggin[layer][:].rearrange("(pp k) ch -> pp (k ch)", k=K2),
                    acc[:])
                nc.gpsimd.collective_compute(
                    "AllGather", mybir.AluOpType.bypass,
                    replica_groups=[list(range(N_CORES))],
                    ins=[aggin[layer][:]], outs=[aggout[layer][:]])

            compact(0)

            # ---- layer-2 tables: h2 = h1 + agg1
            def h_view(t, j0, w):
                # [N_TOT, 5] -> [25, w]: partition (g*5+ch), free n'
                return t[:, :].rearrange("(ng five) ch -> (five ch) ng", five=5)[
                    :, j0 : j0 + w]

            def rhs2(j0, w):
                ht = tp.tile([25, 512], F32, tag="ht")
                at = tp.tile([25, 512], F32, tag="at")
                nc.sync.dma_start(ht[:, :w], h_view(h1d, j0, w))
                nc.sync.dma_start(at[:, :w], h_view(aggout[0], j0, w))
                nc.vector.tensor_tensor(out=ht[:, :w], in0=ht[:, :w], in1=at[:, :w],
                                        op=mybir.AluOpType.add)
                return ht[:, :w]

            def evac2(j0, w, ps):
                for g in range(5):
                    m0 = g * 20
                    nc.sync.dma_start(tview(BD[1], g, j0, w, 0, 10), ps[m0 + 10 : m0 + 20, :w])

            table_pass(1, rhs2, 25, lhsT2_s, bias2_s, 20, evac2)

            # ---- celltab2: CT2 = CT1[:,10:20] + gather(aggout[0])@w2
            HALF = NCALL // 2
            for half in range(2):
                m0 = half * HALF
                mN = NCALL - m0 if half == 1 else HALF
                cidx = bp.tile([128, mN], I32, tag="cidx", name="cidx")
                nc.sync.dma_start(cidx[:], aggidx[:, m0 : m0 + mN])
                gag = bp.tile([128, mN * 5], F32, tag="gag", name="gag")
                for m in range(mN):
                    nc.gpsimd.indirect_dma_start(
                        out=gag[:, m * 5 : (m + 1) * 5],
                        out_offset=None, in_=aggout[0][:],
                        in_offset=bass.IndirectOffsetOnAxis(
                            ap=cidx[:, m : m + 1], axis=0))
                u2 = bp.tile([128, mN * 10], F32, tag="u2c", name="u2c")
                dvr = CT1[m0 * 128 : (m0 + mN) * 128, 10:20].rearrange(
                    "(x pp) ch -> pp x ch", pp=128)
                nc.sync.dma_start(u2[:].rearrange(
                    "pp (x ch) -> pp x ch", ch=10), dvr)
                tmp = bp.tile([128, mN * 10], F32, tag="c2t", name="c2t")
                for c in range(5):
                    gb = gag[:].rearrange("pp (x ch) -> pp x ch", ch=5)[
                        :, :, c : c + 1].broadcast_to([128, mN, 10])
                    wb = wmix2_s[:, c * 10 : c * 10 + 10].unsqueeze(
                        1).broadcast_to([128, mN, 10])
                    nc.vector.tensor_tensor(
                        out=tmp[:].rearrange("pp (x ch) -> pp x ch", ch=10),
                        in0=gb, in1=wb, op=mybir.AluOpType.mult)
                    nc.vector.tensor_tensor(out=u2[:], in0=u2[:], in1=tmp[:],
                                            op=mybir.AluOpType.add)
                dvw = CT2[m0 * 128 : (m0 + mN) * 128, 0:10].rearrange(
                    "(x pp) ch -> pp x ch", pp=128)
                nc.sync.dma_start(dvw, u2[:].rearrange(
                    "pp (x ch) -> pp x ch", ch=10))

            edge_stream(1)
            compact(1)

            if debug:
                nc.sync.dma_start(dbg_h1[:], h1d[:])
                nc.sync.dma_start(dbg_ac[:], AC[0][:])
                nc.sync.dma_start(dbg_bd[:], BD[0][:])
                nc.sync.dma_start(dbg_agc[:], aggcell[0][: NCELL_TOT + 1, :])
                nc.sync.dma_start(dbg_ago[:], aggout[0][:])
                nc.sync.dma_start(dbg_ac2[:], AC[1][:])
                nc.sync.dma_start(dbg_bd2[:], BD[1][:])
                nc.sync.dma_start(dbg_ago2[:], aggout[1][:])

            # ---- final: h3 = h1 + agg1 + agg2; out = h3 @ lin2 + b
            for j in range(NGC):
                j0 = j * 512
                w = min(512, NG - j0)
                ht = tp.tile([25, 512], F32, tag="fht")
                at = tp.tile([25, 512], F32, tag="fat")
                nc.sync.dma_start(ht[:, :w], h_view(h1d, j0, w))
                nc.sync.dma_start(at[:, :w], h_view(aggout[0], j0, w))
                nc.vector.tensor_tensor(out=ht[:, :w], in0=ht[:, :w], in1=at[:, :w],
                                        op=mybir.AluOpType.add)
                nc.sync.dma_start(at[:, :w], h_view(aggout[1], j0, w))
                nc.vector.tensor_tensor(out=ht[:, :w], in0=ht[:, :w], in1=at[:, :w],
                                        op=mybir.AluOpType.add)
                ps = tpp.tile([128, 512], F32, space="PSUM", tag="fps")
                nc.tensor.matmul(ps[:10, :w], lhsT=biaso_s[:],
                                 rhs=ones_s[:, :w], start=True, stop=False)
                nc.tensor.matmul(ps[:10, :w], lhsT=lhsTo_s[:], rhs=ht[:, :w],
                                 start=False, stop=True)
                osb = tp.tile([10, 512], F32, tag="osb")
                nc.vector.tensor_copy(osb[:, :w], ps[:10, :w])
                for g in range(5):
                    nc.sync.dma_start(
                        out[:, :].rearrange("(ng five) ch -> ch ng five", five=5)[
                            :, j0 : j0 + w, g],
                        osb[g * 2 : g * 2 + 2, :w])

    nc.finalize()
    return nc


# ----------------------------------------------------------------------------
# weights packing
# ----------------------------------------------------------------------------
def _host_arrays(p, inputs, n_nodes):
    N_TOT, NG, SP = p.N_TOT, p.NG, p.SP
    x = np.asarray(inputs["x"], np.float32)
    lin1_w = np.asarray(inputs["lin1_w"], np.float32)
    lin1_b = np.asarray(inputs["lin1_b"], np.float32)
    lin2_w = np.asarray(inputs["lin2_w"], np.float32)
    lin2_b = np.asarray(inputs["lin2_b"], np.float32)
    wf1 = np.asarray(inputs["conv1_wf"], np.float32)
    bf1 = np.asarray(inputs["conv1_bf"], np.float32)
    ws1 = np.asarray(inputs["conv1_ws"], np.float32)
    bs1 = np.asarray(inputs["conv1_bs"], np.float32)
    wf2 = np.asarray(inputs["conv2_wf"], np.float32)
    bf2 = np.asarray(inputs["conv2_bf"], np.float32)
    ws2 = np.asarray(inputs["conv2_ws"], np.float32)
    bs2 = np.asarray(inputs["conv2_bs"], np.float32)

    rel2orig = np.full(N_TOT, -1, dtype=np.int64)
    rel2orig[p.orig2rel] = np.arange(n_nodes)
    valid = rel2orig >= 0
    x_rel = np.zeros((N_TOT, 2), np.float32)
    x_rel[valid] = x[rel2orig[valid]]

    xT = _pack_T(x_rel, p, 2)  # [10, NG]

    # layer-1 combined projections: [A'|C'|B|D|h] (25 ch per group)
    W1 = np.concatenate([
        lin1_w @ wf1[0:CH], lin1_w @ ws1[0:CH],
        lin1_w @ wf1[CH : 2 * CH], lin1_w @ ws1[CH : 2 * CH],
        lin1_w,
    ], axis=1)  # [2, 25]
    b1 = np.concatenate([
        lin1_b @ wf1[0:CH] + bf1, lin1_b @ ws1[0:CH] + bs1,
        lin1_b @ wf1[CH : 2 * CH], lin1_b @ ws1[CH : 2 * CH],
        lin1_b,
    ])  # [25]
    lhsT1 = np.zeros((10, 125), np.float32)
    bias1 = np.zeros((1, 125), np.float32)
    for g in range(5):
        lhsT1[g * 2 : g * 2 + 2, g * 25 : (g + 1) * 25] = W1
        bias1[0, g * 25 : (g + 1) * 25] = b1

    W2 = np.concatenate([wf2[0:CH], ws2[0:CH], wf2[CH : 2 * CH], ws2[CH : 2 * CH]],
                        axis=1)  # [5, 20]
    b2 = np.concatenate([bf2, bs2, np.zeros(10, np.float32)])
    lhsT2 = np.zeros((25, 100), np.float32)
    bias2 = np.zeros((1, 100), np.float32)
    for g in range(5):
        lhsT2[g * 5 : (g + 1) * 5, g * 20 : (g + 1) * 20] = W2
        bias2[0, g * 20 : (g + 1) * 20] = b2

    lhsTo = np.zeros((25, 10), np.float32)
    biaso = np.zeros((1, 10), np.float32)
    for g in range(5):
        lhsTo[g * 5 : (g + 1) * 5, g * 2 : (g + 1) * 2] = lin2_w
        biaso[0, g * 2 : (g + 1) * 2] = lin2_b

    wpat = np.zeros((128, 20), np.float32)
    wpat[:, 0:5] = wf1[2 * CH]
    wpat[:, 5:10] = ws1[2 * CH]
    wpat[:, 10:15] = wf2[2 * CH]
    wpat[:, 15:20] = ws2[2 * CH]

    lseg = np.zeros((128, 8), np.float32)
    for cc in range(CPC):
        lseg[cc * CELL : (cc + 1) * CELL, cc] = 1.0
    onesd = np.ones((1, 512), np.float32)
    lexp = np.zeros((CPC, 128), np.float32)
    for r in range(128):
        lexp[r // CELL, r] = 1.0
    # cell-pass weights: out ch = [A1(5) C1(5) U2A(5) U2C(5)] = 20 per block
    BLK = 4
    Wc = np.concatenate([lin1_w @ wf1[0:CH], lin1_w @ ws1[0:CH],
                         lin1_w @ wf2[0:CH], lin1_w @ ws2[0:CH]], axis=1)
    bcv = np.concatenate([lin1_b @ wf1[0:CH] + bf1, lin1_b @ ws1[0:CH] + bs1,
                          lin1_b @ wf2[0:CH] + bf2, lin1_b @ ws2[0:CH] + bs2])
    lhsTc = np.zeros((2 * BLK, BLK * 20), np.float32)
    biasc = np.zeros((1, BLK * 20), np.float32)
    for b in range(BLK):
        for xc in range(2):
            lhsTc[xc * BLK + b, b * 20 : (b + 1) * 20] = Wc[xc]
        biasc[0, b * 20 : (b + 1) * 20] = bcv
    # agg->AC2 transform rows: wmix2[:, c*10 : c*10+10] = [wf2[c,:] | ws2[c,:]]
    wmix2 = np.zeros((128, 50), np.float32)
    for c2 in range(CH):
        wmix2[:, c2 * 10 : c2 * 10 + 5] = wf2[c2]
        wmix2[:, c2 * 10 + 5 : c2 * 10 + 10] = ws2[c2]

    shared = {
        "xT": xT, "lhsT1": lhsT1, "bias1": bias1, "lhsT2": lhsT2, "bias2": bias2,
        "lhsTo": lhsTo, "biaso": biaso, "wpat": wpat, "lseg": lseg,
        "onesd": onesd, "lhsTc": lhsTc, "biasc": biasc, "lexp": lexp,
        "wmix2": wmix2,
    }
    in_maps = []
    for k in range(N_CORES):
        c = p.core[k]
        m = dict(shared)
        m["srcg"] = c["src_g"]
        m["eg"] = c["e_g"]
        m["fcd"] = c["fc"]
        m["cmask"] = c["cmask"]
        m["aggidx"] = c["aggidx"]
        # xcell pack [8, NCELL512/4]: row xc*4+b, col i*128+pp ;
        # cell = i*512 + b*128 + pp
        cl = c["cell_lin"]
        xcell = np.zeros((p.NCELL512, 2), np.float32)
        real = cl < p.N_TOT
        xcell[real] = x_rel[cl[real]]
        ncit = p.NCELL512 // 512
        xcv = xcell.reshape(ncit, 4, 128, 2)
        m["xcTP"] = np.ascontiguousarray(
            np.transpose(xcv, (3, 1, 0, 2)).reshape(8, -1))
        in_maps.append(m)
    return in_maps, rel2orig


# ----------------------------------------------------------------------------
# entry point
# ----------------------------------------------------------------------------
_CACHE = {}


def kernel_impl(inputs, n_nodes):
    ei = np.asarray(inputs["edge_index"])
    ea = np.asarray(inputs["edge_attr"])
    key = (ei.shape[1], n_nodes)
    p = _prepare(ei, ea, n_nodes)
    if key not in _CACHE:
        _CACHE[key] = _build(p)
    nc = _CACHE[key]
    in_maps, rel2orig = _host_arrays(p, inputs, n_nodes)
    res = run_bass_kernel_spmd(nc, in_maps, core_ids=list(range(N_CORES)))
    out_rel = res.results[0]["out"]
    return np.ascontiguousarray(out_rel[p.orig2rel]).astype(np.float32)


def kernel(**inputs):
    return kernel_impl(inputs, 100000)

